# revision 1
# baseline (speedup 1.0000x reference)
"""Multi-head attention (B=2, S=2048, D=1024, H=16) on 8 Trainium2 NeuronCores.

Sharding: core i handles batch b = i//4 and head-group hg = i%4 (4 heads,
256 channels).  Per-head Q/K projection weights are replicated; the fc layer
is sharded over its contraction dim (each core contributes a partial y that
the host sums per batch).  Wv is folded into Wfc on the host (exact, since
softmax rows sum to 1 the bv contribution folds into bfc).

Device algorithm per core (all matmuls f32r, psum f32):
  - heads processed as PAIRS: head A on SBUF partitions 0-63, head B on
    64-127, so the K=64 score matmuls run as concurrent 64-row PE tiles
  - q'T/k'T projections: head B via a row-tiled M=128 matmul with
    block-diagonal weights, head A plain M=64 (scale 1/sqrt(64) folded in Wq)
  - scores computed transposed: S_t[k,q] = k'T_slice.T @ q'T (k on
    partitions); exp on ScalarE (no max-subtraction; |scores| <~ 2 so exp
    is safe), one N=1024 activation per k-tile covering both heads
  - AV: lhsT = [V_h | ones] (65 cols) so row 64 of the psum accumulator is
    the softmax denominator; accumulators are evacuated to SBUF and
    normalization is deferred one round (reciprocal -> ones-matmul
    partition-broadcast -> multiply); head B's normalized oT additionally
    goes through an identity-shift matmul to partitions 64-127
  - fc: y[s,c] accumulated over the two head-pairs with K=128 matmuls,
    emitted as single-matmul closures interleaved one-per-k-tile into the
    next q-window's attention so they hide under the ScalarE exp stream
"""


import sys

import numpy as np

if "/opt/trn_rl_repo" not in sys.path:
    sys.path.insert(0, "/opt/trn_rl_repo")

HEAD = 16
B, S, D = 2, 2048, 1024
HD = 64
HPC = 4          # heads per core
CH = HPC * HD    # channels per core
N_CORES = 8

_CACHE = {}
LAST_RESULTS = None


def _build():
    import concourse.tile as tile
    from concourse import bacc, mybir

    f32 = mybir.dt.float32
    f32r = mybir.dt.float32r
    EXP = mybir.ActivationFunctionType.Exp

    nc = bacc.Bacc("TRN2", target_bir_lowering=False, debug=False,
                   num_devices=N_CORES)

    # unused internal tensor whose name varies per retry: changes the BIR
    # content hash so a retry never reuses a possibly-corrupt cached NEFF
    nonce = _CACHE.get("nonce", 0)
    if nonce:
        nc.dram_tensor(f"retry_nonce_{nonce}", [1, 1], mybir.dt.float32)

    qt_d = nc.dram_tensor("qt", [CH, S], f32r, kind="ExternalInput")
    kt_d = nc.dram_tensor("kt", [CH, S], f32r, kind="ExternalInput")
    v1_d = nc.dram_tensor("v1", [S, 65 * HPC], f32r, kind="ExternalInput")
    wq_d = nc.dram_tensor("wqt", [2 * HD, 2 * HD], f32r, kind="ExternalInput")
    wk_d = nc.dram_tensor("wkt", [2 * HD, 2 * HD], f32r, kind="ExternalInput")
    bq_d = nc.dram_tensor("bq", [2 * HD, 1], f32, kind="ExternalInput")
    bk_d = nc.dram_tensor("bk", [2 * HD, 1], f32, kind="ExternalInput")
    wf_d = nc.dram_tensor("wfct", [CH, D], f32r, kind="ExternalInput")
    on_d = nc.dram_tensor("ones", [1, HD], f32r, kind="ExternalInput")
    ish_d = nc.dram_tensor("ishift", [HD, 2 * HD], f32r, kind="ExternalInput")
    y_d = nc.dram_tensor("y", [S, D], f32, kind="ExternalOutput")

    with tile.TileContext(nc) as tc, nc.allow_low_precision(
            reason="f32r tiles feed tensor-engine matmuls; psum stays f32"):
        with (
            tc.tile_pool(name="consts", bufs=1) as consts,
            tc.tile_pool(name="vpool", bufs=1) as vpool,
            tc.tile_pool(name="qk_in", bufs=2) as qk_in,
            tc.tile_pool(name="qk_proj", bufs=2) as qk_proj,
            tc.tile_pool(name="ot", bufs=1) as otp,
            tc.tile_pool(name="exp", bufs=3) as expp,
            tc.tile_pool(name="small", bufs=2) as small,
            tc.tile_pool(name="ysb", bufs=3) as ysb,
            tc.tile_pool(name="ps_score", bufs=2, space="PSUM") as ps_score,
            tc.tile_pool(name="ps_av", bufs=2, space="PSUM") as ps_av,
            tc.tile_pool(name="ps_misc", bufs=1, space="PSUM") as ps_misc,
        ):
            # ---------------- constants ----------------
            # only wk/wq gate the first projection; the rest can trail the
            # first input chunks
            wq_s = consts.tile([2 * HD, 2 * HD], f32r, tag="wq")
            wk_s = consts.tile([2 * HD, 2 * HD], f32r, tag="wk")
            bq_s = consts.tile([2 * HD, 1], f32, tag="bq")
            bk_s = consts.tile([2 * HD, 1], f32, tag="bk")
            ones_s = consts.tile([65, HD], f32r, tag="ones")
            ish_s = consts.tile([HD, 2 * HD], f32r, tag="ishift")

            def emit_late_consts():
                nc.sync.dma_start(out=bk_s, in_=bk_d[:, :])
                nc.sync.dma_start(out=bq_s, in_=bq_d[:, :])
                nc.sync.dma_start(out=ones_s[64:65, :], in_=on_d[:, :])
                nc.sync.dma_start(out=ish_s, in_=ish_d[:, :])
            # ---------------- projections ----------------
            # chunked input DMAs so the first proj matmul starts early;
            # j=0 inputs first, then v1 (needed from the first av), then the
            # remaining heads, then wfct (needed only by fc, much later)
            qp_s, kp_s = [], []
            v1_s = []
            wf_s = []
            deferred_qt = []

            def emit_proj(p):
                # head pair p: head 2p on partitions 0-63, head 2p+1 on
                # partitions 64-127 (concurrent 64x64 PE tiles T0 / T10)
                qt_t = qk_in.tile([2 * HD, S], f32r, tag="qt_in",
                                  name=f"qt_in{p}")
                kt_t = qk_in.tile([2 * HD, S], f32r, tag="kt_in",
                                  name=f"kt_in{p}")
                if p == 0:
                    # critical-path order: the first k-proj matmul needs only
                    # wk + kt chunk 0; scores consume kp chunk-by-chunk, but
                    # qt chunks 1-3 are not needed until the second q-window,
                    # so defer them until after the v1 loads
                    nc.sync.dma_start(out=wk_s, in_=wk_d[:, :])
                    nc.sync.dma_start(out=kt_t[:, 0:512],
                                      in_=kt_d[0:128, 0:512])
                    nc.sync.dma_start(out=wq_s, in_=wq_d[:, :])
                    nc.sync.dma_start(out=qt_t[:, 0:512],
                                      in_=qt_d[0:128, 0:512])
                    emit_late_consts()
                    for c in range(1, S // 512):
                        sl = slice(512 * c, 512 * c + 512)
                        nc.sync.dma_start(out=kt_t[:, sl],
                                          in_=kt_d[0:128, sl])
                    for c in range(1, S // 512):
                        sl = slice(512 * c, 512 * c + 512)
                        nc.sync.dma_start(out=qt_t[:, sl],
                                          in_=qt_d[0:128, sl])
                else:
                    for c in range(S // 512):
                        sl = slice(512 * c, 512 * c + 512)
                        nc.sync.dma_start(out=kt_t[:, sl],
                                          in_=kt_d[128 * p:128 * p + 128, sl])
                        nc.sync.dma_start(out=qt_t[:, sl],
                                          in_=qt_d[128 * p:128 * p + 128, sl])
                qp = qk_proj.tile([2 * HD, S], f32r, tag="qp", name=f"qp{p}")
                kp = qk_proj.tile([2 * HD, S], f32r, tag="kp", name=f"kp{p}")
                for qb in range(S // 512):
                    sl = slice(512 * qb, 512 * qb + 512)
                    # head B: row-tiled M=128 matmul with block-diag weights
                    # (only rows 64-127 valid); head A: plain M=64 matmul in
                    # a separate psum slot (same-bank double-write is a HW
                    # runtime error)
                    pk1 = ps_misc.tile([128, 512], f32, tag="misc",
                                       name=f"pk1{p}_{qb}")
                    nc.tensor.matmul(pk1, wk_s[64:128, :],
                                     kt_t[64:128, sl], start=True, stop=True)
                    nc.vector.tensor_scalar_add(kp[64:128, sl],
                                                pk1[64:128, :], bk_s[64:128])
                    pk2 = ps_misc.tile([128, 512], f32, tag="misc",
                                       name=f"pk2{p}_{qb}")
                    nc.tensor.matmul(pk2[0:64, :], wk_s[0:64, 0:64],
                                     kt_t[0:64, sl], start=True, stop=True)
                    nc.vector.tensor_scalar_add(kp[0:64, sl],
                                                pk2[0:64, :], bk_s[0:64])
                    pq1 = ps_misc.tile([128, 512], f32, tag="misc",
                                       name=f"pq1{p}_{qb}")
                    nc.tensor.matmul(pq1, wq_s[64:128, :],
                                     qt_t[64:128, sl], start=True, stop=True)
                    nc.vector.tensor_scalar_add(qp[64:128, sl],
                                                pq1[64:128, :], bq_s[64:128])
                    pq2 = ps_misc.tile([128, 512], f32, tag="misc",
                                       name=f"pq2{p}_{qb}")
                    nc.tensor.matmul(pq2[0:64, :], wq_s[0:64, 0:64],
                                     qt_t[0:64, sl], start=True, stop=True)
                    nc.vector.tensor_scalar_add(qp[0:64, sl],
                                                pq2[0:64, :], bq_s[0:64])
                qp_s.append(qp)
                kp_s.append(kp)

            emit_proj(0)
            for t in range(S // 128):
                v1t = vpool.tile([128, 65 * HPC], f32r, tag=f"v1_{t}",
                                 name=f"v1_{t}")
                nc.sync.dma_start(out=v1t, in_=v1_d[128 * t:128 * t + 128, :])
                v1_s.append(v1t)
            for fn in deferred_qt:
                fn()
            emit_proj(1)
            for pr in range(2):
                wfj = consts.tile([2 * HD, D], f32r, tag=f"wf{pr}",
                                  name=f"wf{pr}")
                nc.sync.dma_start(out=wfj,
                                  in_=wf_d[128 * pr:128 * pr + 128, :])
                wf_s.append(wfj)

            # ---------------- attention + interleaved fc ----------------
            oT_s = []
            for pr in range(2):
                oT = otp.tile([2 * HD, S], f32r, tag=f"oT{pr}", name=f"oT{pr}")
                oT_s.append(oT)

            NQB = S // 512           # outer q windows (512 wide)
            NKT = S // 128           # k tiles

            def emit_norm(p, qb, oc_t):
                # oc_t: sbuf [65, 1024] copy of the av accumulators for the
                # head pair (head 2p cols 0:512, head 2p+1 cols 512:1024;
                # row 64 = denominators). Normalize into the oT pair tile;
                # head B additionally goes through an identity-shift matmul
                # to land on partitions 64-127 (so fc can contract K=128).
                rsb = small.tile([65, 1024], f32r, tag="r",
                                 name=f"r{p}_{qb}")
                nc.vector.reciprocal(out=rsb[64:65, :], in_=oc_t[64:65, :])
                q0 = 512 * qb
                # head A (2p): normalize straight into rows 0-63
                rbpa = ps_misc.tile([HD, 512], f32, tag="rbp",
                                    name=f"rbpa{p}_{qb}")
                nc.tensor.matmul(rbpa, ones_s[64:65, :], rsb[64:65, 0:512],
                                 start=True, stop=True)
                nc.vector.tensor_mul(oT_s[p][0:64, q0:q0 + 512],
                                     rbpa, oc_t[0:64, 0:512])
                # head B (2p+1): normalize into a temp, shift to rows 64-127
                rbpb = ps_misc.tile([HD, 512], f32, tag="rbp",
                                    name=f"rbpb{p}_{qb}")
                nc.tensor.matmul(rbpb, ones_s[64:65, :], rsb[64:65, 512:1024],
                                 start=True, stop=True)
                oTb = small.tile([HD, 512], f32r, tag="oTb",
                                 name=f"oTb{p}_{qb}")
                nc.vector.tensor_mul(oTb, rbpb, oc_t[0:64, 512:1024])
                shp = ps_misc.tile([128, 512], f32, tag="rbp",
                                   name=f"shp{p}_{qb}")
                nc.tensor.matmul(shp, ish_s, oTb, start=True, stop=True)
                nc.vector.tensor_copy(oT_s[p][64:128, q0:q0 + 512],
                                      shp[64:128, :])

            # fc is emitted as single-matmul closures popped one per k-tile
            # iteration, so they never displace more than ~213ns of the
            # score->exp->av pipeline at a time.
            fc_state = {}

            def emit_fc_op(st, cb, pr, pool, tag):
                if cb == 0 and pr == 0:
                    fc_state["y_sb"] = ysb.tile([128, D], f32, tag="y",
                                                name=f"y{st}")
                if pr == 0:
                    fc_state["yp"] = pool.tile([128, 512], f32, tag=tag,
                                               name=f"yp{st}_{cb}")
                yp = fc_state["yp"]
                nc.tensor.matmul(
                    yp,
                    oT_s[pr][:, 128 * st:128 * st + 128],
                    wf_s[pr][:, 512 * cb:512 * cb + 512],
                    start=(pr == 0), stop=(pr == 1))
                if pr == 1:
                    y_sb = fc_state["y_sb"]
                    nc.vector.tensor_copy(y_sb[:, 512 * cb:512 * cb + 512], yp)
                    if cb == D // 512 - 1:
                        nc.sync.dma_start(
                            out=y_d[128 * st:128 * st + 128, :], in_=y_sb)

            # last q-window: pair-0 partials staged to SBUF during the final
            # attention round, pair-1 matmul + combine in the drain
            ya_st = {}

            def emit_fcA_op(st, cb):
                ypa = ps_misc.tile([128, 512], f32, tag="misc",
                                   name=f"ypa{st}_{cb}")
                nc.tensor.matmul(ypa,
                                 oT_s[0][:, 128 * st:128 * st + 128],
                                 wf_s[0][:, 512 * cb:512 * cb + 512],
                                 start=True, stop=True)
                ya = ysb.tile([128, 512], f32, tag="ya", bufs=8,
                              name=f"ya{st}_{cb}")
                nc.vector.tensor_copy(ya, ypa)
                ya_st[(st, cb)] = ya

            def emit_fcB_op(st, cb):
                if cb == 0:
                    fc_state["y_sb"] = ysb.tile([128, D], f32, tag="y",
                                                name=f"y{st}")
                # rotate three psum slots (2x score + the idle misc bank) so
                # the drain-phase matmul/copy chains pipeline deeper
                if (2 * st + cb) % 3 == 2:
                    ypb = ps_misc.tile([128, 512], f32, tag="misc",
                                       name=f"ypb{st}_{cb}")
                else:
                    ypb = ps_score.tile([128, 512], f32, tag="score",
                                        name=f"ypb{st}_{cb}")
                nc.tensor.matmul(ypb,
                                 oT_s[1][:, 128 * st:128 * st + 128],
                                 wf_s[1][:, 512 * cb:512 * cb + 512],
                                 start=True, stop=True)
                y_sb = fc_state["y_sb"]
                nc.vector.tensor_add(y_sb[:, 512 * cb:512 * cb + 512],
                                     ya_st[(st, cb)], ypb)
                if cb == D // 512 - 1:
                    nc.sync.dma_start(
                        out=y_d[128 * st:128 * st + 128, :], in_=y_sb)

            pending_norm = None
            fc_queue = []
            for qb in range(NQB):
                for p in range(2):
                    o_ps = []
                    for half in range(2):
                        o = ps_av.tile([65, 512], f32, tag="av",
                                       name=f"o{p}_{qb}_{half}")
                        o_ps.append(o)
                    q0 = 512 * qb
                    qa = qp_s[p][0:64, q0:q0 + 512]
                    qb_ = qp_s[p][64:128, q0:q0 + 512]
                    for kt in range(NKT):
                        ks = slice(128 * kt, 128 * kt + 128)
                        sc = ps_score.tile([128, 1024], f32, tag="score",
                                           name=f"sc{p}_{qb}_{kt}")
                        nc.tensor.matmul(sc[:, 0:512], kp_s[p][0:64, ks], qa,
                                         start=True, stop=True)
                        nc.tensor.matmul(sc[:, 512:1024],
                                         kp_s[p][64:128, ks], qb_,
                                         start=True, stop=True)
                        ex = expp.tile([128, 1024], f32r, tag="exp",
                                       name=f"ex{p}_{qb}_{kt}")
                        nc.scalar.activation(out=ex, in_=sc, func=EXP)
                        va = v1_s[kt][:, 65 * 2 * p:65 * 2 * p + 65]
                        vb = v1_s[kt][:, 65 * (2 * p + 1):65 * (2 * p + 1) + 65]
                        nc.tensor.matmul(o_ps[0], va, ex[:, 0:512],
                                         start=(kt == 0), stop=(kt == NKT - 1))
                        nc.tensor.matmul(o_ps[1], vb, ex[:, 512:1024],
                                         start=(kt == 0), stop=(kt == NKT - 1))
                        if kt == 2 and pending_norm is not None:
                            emit_norm(*pending_norm)
                            pending_norm = None
                            if qb == NQB - 1 and p == 1:
                                # pair-0 of the last window is normalized now;
                                # its fc partials can overlap this last round
                                for st_ in range(4 * qb, 4 * qb + 4):
                                    for cb_ in range(D // 512):
                                        fc_queue.append(
                                            lambda st=st_, cb=cb_:
                                                emit_fcA_op(st, cb))
                        if fc_queue:
                            fc_queue.pop(0)()
                    # evacuate the accumulators to SBUF quickly so the av
                    # psum slots free up; normalization is deferred
                    oc_t = small.tile([65, 1024], f32, tag="oc", bufs=4,
                                      name=f"oc{p}_{qb}")
                    nc.vector.tensor_copy(oc_t[:, 0:512], o_ps[0])
                    nc.vector.tensor_copy(oc_t[:, 512:1024], o_ps[1])
                    pending_norm = (p, qb, oc_t)
                # fc for this q-window needs both pairs' norms done
                emit_norm(*pending_norm)
                pending_norm = None
                # earlier windows' fc pops during later attention (misc psum
                # slot); the last window is split: pair-0 partials pop during
                # the final round, pair-1 + combine drain at the end
                if qb < NQB - 1:
                    for st in range(4 * qb, 4 * qb + 4):
                        for cb in range(D // 512):
                            for pr in range(2):
                                fc_queue.append(
                                    lambda st=st, cb=cb, pr=pr:
                                        emit_fc_op(st, cb, pr, ps_misc,
                                                   "misc"))
                # (last window's fcA ops are enqueued mid-round, above)
            while fc_queue:
                fc_queue.pop(0)()
            for st in range(S // 128 - 4, S // 128):
                for cb in range(D // 512):
                    emit_fcB_op(st, cb)

    nc.compile()
    return nc


def _prep(query, key, value, Wq, bq, Wk, bk, Wv, bv, Wfc, bfc):
    """Host-side sharding / layout prep. Returns (in_maps, bfc_eff)."""
    query = np.asarray(query, dtype=np.float32)
    key = np.asarray(key, dtype=np.float32)
    value = np.asarray(value, dtype=np.float32)
    Wq = np.asarray(Wq, np.float32); bq = np.asarray(bq, np.float32)
    Wk = np.asarray(Wk, np.float32); bk = np.asarray(bk, np.float32)
    Wv = np.asarray(Wv, np.float32); bv = np.asarray(bv, np.float32)
    Wfc = np.asarray(Wfc, np.float32); bfc = np.asarray(bfc, np.float32)

    scale = np.float32(1.0 / np.sqrt(HD))
    wq_t = np.ascontiguousarray(Wq.T) * scale        # [d, e], scale folded
    bq_sc = (bq * scale).reshape(HD, 1).astype(np.float32)
    wk_t = np.ascontiguousarray(Wk.T)
    bk_c = bk.reshape(HD, 1).astype(np.float32)
    # block-diagonal for head-pair packing: head A reads [0:64, 0:64],
    # head B reads rows 64:128 as [zeros | w] (row-tiled M=128 matmul)
    z = np.zeros((HD, HD), np.float32)
    wq_t2 = np.ascontiguousarray(np.block([[wq_t, z], [z, wq_t]]))
    wk_t2 = np.ascontiguousarray(np.block([[wk_t, z], [z, wk_t]]))
    bq_2 = np.ascontiguousarray(np.vstack([bq_sc, bq_sc]))
    bk_2 = np.ascontiguousarray(np.vstack([bk_c, bk_c]))

    # fold Wv / bv into fc
    A = np.empty((D, D), np.float32)
    bfc_eff = bfc.astype(np.float32).copy()
    for h in range(HEAD):
        Wfc_h = Wfc[:, HD * h:HD * h + HD]
        A[:, HD * h:HD * h + HD] = Wfc_h @ Wv
        bfc_eff += Wfc_h @ bv
    At = np.ascontiguousarray(A.T)                    # [ch, c]

    ishift = np.zeros((HD, 2 * HD), np.float32)
    ishift[np.arange(HD), HD + np.arange(HD)] = 1.0

    qT = np.ascontiguousarray(query.transpose(0, 2, 1))   # [B, D, S]
    kT = np.ascontiguousarray(key.transpose(0, 2, 1))

    in_maps = []
    for core in range(N_CORES):
        b, hg = core // 4, core % 4
        ch0 = CH * hg
        v1 = np.empty((S, 65 * HPC), np.float32)
        for j in range(HPC):
            v1[:, 65 * j:65 * j + 64] = value[b][:, ch0 + HD * j:ch0 + HD * j + HD]
            v1[:, 65 * j + 64] = 1.0
        in_maps.append({
            "qt": np.ascontiguousarray(qT[b][ch0:ch0 + CH]),
            "kt": np.ascontiguousarray(kT[b][ch0:ch0 + CH]),
            "v1": v1,
            "wqt": wq_t2,
            "wkt": wk_t2,
            "bq": bq_2,
            "bk": bk_2,
            "wfct": np.ascontiguousarray(At[ch0:ch0 + CH]),
            "ones": np.ones((1, HD), np.float32),
            "ishift": ishift,
        })
    return in_maps, bfc_eff


def _run_once(inputs):
    global LAST_RESULTS
    from concourse.bass_utils import run_bass_kernel_spmd

    if "nc" not in _CACHE:
        _CACHE["nc"] = _build()
    nc = _CACHE["nc"]

    in_maps, bfc_eff = _prep(**inputs)
    res = run_bass_kernel_spmd(nc, in_maps, core_ids=list(range(N_CORES)))
    LAST_RESULTS = res

    out = np.empty((B, S, D), np.float32)
    for b in range(B):
        acc = res.results[4 * b]["y"].astype(np.float32).copy()
        for hg in range(1, 4):
            acc += res.results[4 * b + hg]["y"]
        out[b] = acc + bfc_eff
    return out


def kernel(**inputs) -> np.ndarray:
    last_exc = None
    for attempt in range(3):
        try:
            out = _run_once(inputs)
            amax = float(np.abs(out).max())
            if np.isfinite(out).all() and 1e-6 < amax < 1e3:
                return out
            raise RuntimeError(f"implausible kernel output (absmax={amax})")
        except Exception as e:  # noqa: BLE001 - retry transient HW failures
            last_exc = e
            _CACHE.pop("nc", None)
            _CACHE["nonce"] = attempt + 1
    raise last_exc



# revision 14
# speedup vs baseline: 1.0808x; 1.0808x over previous
"""Multi-head attention (B=2, S=2048, D=1024, H=16) on 8 Trainium2 NeuronCores.

Sharding: core i handles batch b = i//4 and head-group hg = i%4 (4 heads,
256 channels).  The fc layer is sharded over its contraction dim; each core
emits a bf16 partial y that the host sums per batch.  Wv/bv are folded into
Wfc/bfc on the host (exact: softmax rows sum to 1).

Algebraic folds (host, weights-only):
  - q-projection is folded into the k side: score = q_raw . (M k_raw) with
    M = Wq^T Wk, so only k needs an on-device projection (k~ = M k).
  - bias cross-terms: score = (Wq q).(Wk k) + c(k) + d(q) + const; the d(q)
    and const terms cancel in softmax; exp(c(k)/8) is folded into the v/ones
    columns on the host (c(k) = (Wk^T bq).k + bq.bk; exactly 1 when bq=0).

Device pipeline per core (per-head attention, 4 heads):
  - k~ projection: fp8 raw k x fp8 block-diag M columns -> psum -> fp8 k~D
    in DoubleRow layout [32, 2, S] per head (contraction split 64 = 32x2)
  - scores: fp8 DoubleRow matmuls (2 heads per 1024-wide psum tile),
    0.5 cycles/row on the PE
  - exp: split across ScalarE (native Exp -> bf16) and VectorE (Schraudolph
    fast-exp: score*16*log2e + (16256-5.5) converted to int16, bit-cast to
    bf16); both write per-kt ex tiles [128k, 1024] bf16
  - AV transposed: o2[q,66] = sum_kt exT[k,q-tile].T @ v1[k,66] (bf16, ones
    column 64 = E/32 gives the denominator; col 65 pads to an even width)
  - normalize: per-partition reciprocal of col 64 + tensor_scalar multiply
    into a [128,128] staging tile (two heads side by side), then a DMA-XBAR
    transpose to oT[pair][ch,128q] -- which also lands head B on partitions
    64..127, making fc a plain K=128 matmul per pair
  - fc: per (st, cb): two bf16 matmuls (pair 0 start / pair 1 stop) into a
    [128,512] psum, DVE-evacuated to bf16 and DMA'd out
  - AV/norm/fc work of window W is interleaved into window W+1's kt slots so
    the PE, ScalarE and VectorE streams all stay busy
"""

import sys

import numpy as np

if "/opt/trn_rl_repo" not in sys.path:
    sys.path.insert(0, "/opt/trn_rl_repo")

HEAD = 16
B, S, D = 2, 2048, 1024
HD = 64
HPC = 4          # heads per core
CH = HPC * HD    # channels per core
N_CORES = 8
NQB = S // 512   # q windows
NKT = S // 128   # k tiles
NST = S // 128   # output row tiles

LOG2E = float(np.log2(np.e))
ACT_SCALE = 0.125           # 1/sqrt(hd) applied inside exp
FEXP_MUL = 16.0 * LOG2E     # bf16 fast-exp: bits = X*16*log2e + 16256 - sigma
FEXP_ADD = 16256.0 - 5.5
S_O = 32.0                  # oT scale (ones column = E/32)
WF_S = 16.0                 # wf scale
OUT_SCALE = 1.0 / (S_O * WF_S)

_CACHE = {}
LAST_RESULTS = None


def _build():
    import concourse.tile as tile
    from concourse import bacc, mybir

    f32 = mybir.dt.float32
    f8 = mybir.dt.float8e4
    bf16 = mybir.dt.bfloat16
    i16 = mybir.dt.int16
    EXP = mybir.ActivationFunctionType.Exp
    DR = mybir.MatmulPerfMode.DoubleRow
    MULT = mybir.AluOpType.mult

    nc = bacc.Bacc("TRN2", target_bir_lowering=False, debug=False,
                   num_devices=N_CORES)

    nonce = _CACHE.get("nonce", 0)
    if nonce:
        nc.dram_tensor(f"retry_nonce_{nonce}", [1, 1], mybir.dt.float32)

    qd0_d = nc.dram_tensor("qd0", [64, 2 * S], f8, kind="ExternalInput")
    qd1_d = nc.dram_tensor("qd1", [64, 2 * S], f8, kind="ExternalInput")
    kt0_d = nc.dram_tensor("kt0", [128, S], f8, kind="ExternalInput")
    kt1_d = nc.dram_tensor("kt1", [128, S], f8, kind="ExternalInput")
    mqlo_d = nc.dram_tensor("mqlo", [128, 64], f8, kind="ExternalInput")
    mqhi_d = nc.dram_tensor("mqhi", [128, 64], f8, kind="ExternalInput")
    v1_d = nc.dram_tensor("v1", [128, NKT * HPC * 66], bf16,
                          kind="ExternalInput")
    wf_d = nc.dram_tensor("wf", [128, 2 * D], bf16, kind="ExternalInput")
    y_d = nc.dram_tensor("y", [S, D], bf16, kind="ExternalOutput")

    with tile.TileContext(nc) as tc, nc.allow_low_precision(
            reason="fp8/bf16 matmul inputs; psum accumulation stays f32"):
        with (
            tc.tile_pool(name="consts", bufs=1) as consts,
            tc.tile_pool(name="exp", bufs=34) as expp,
            tc.tile_pool(name="stag", bufs=4) as stagp,
            tc.tile_pool(name="rd", bufs=4) as rdp,
            tc.tile_pool(name="ysb", bufs=3) as ysbp,
            tc.tile_pool(name="ps_sc", bufs=2, space="PSUM") as ps_sc,
            tc.tile_pool(name="ps_av", bufs=2, space="PSUM") as ps_av,
            tc.tile_pool(name="ps_misc", bufs=2, space="PSUM") as ps_misc,
        ):
            # ---------------- input tiles ----------------
            mqlo_s = consts.tile([128, 64], f8, tag="mqlo")
            mqhi_s = consts.tile([128, 64], f8, tag="mqhi")
            kt0_s = consts.tile([128, S], f8, tag="kt0")
            kt1_s = consts.tile([128, S], f8, tag="kt1")
            qd_s = [consts.tile([64, 2, S], f8, tag=f"qd{pr}",
                                name=f"qd{pr}") for pr in range(2)]
            v1_s = consts.tile([128, NKT, HPC, 66], bf16, tag="v1")
            wf_s = consts.tile([128, 2, D], bf16, tag="wf")
            ktd_s = [consts.tile([64, 2, S], f8, tag=f"ktd{pr}",
                                 name=f"ktd{pr}") for pr in range(2)]
            oT_s = [consts.tile([128, S], bf16, tag=f"oT{pr}",
                                name=f"oT{pr}") for pr in range(2)]

            # critical-path first: proj weights + k chunks, then q, v, wf
            nc.sync.dma_start(out=mqlo_s, in_=mqlo_d[:, :])
            nc.sync.dma_start(out=mqhi_s, in_=mqhi_d[:, :])
            for c in range(4):
                sl = slice(512 * c, 512 * c + 512)
                nc.sync.dma_start(out=kt0_s[:, sl], in_=kt0_d[:, sl])
                nc.sync.dma_start(out=kt1_s[:, sl], in_=kt1_d[:, sl])
            nc.sync.dma_start(out=qd_s[0], in_=qd0_d[:, :])
            nc.sync.dma_start(out=qd_s[1], in_=qd1_d[:, :])
            nc.sync.dma_start(out=v1_s, in_=v1_d[:, :])
            nc.sync.dma_start(out=wf_s, in_=wf_d[:, :])

            # ---------------- k~ projection ----------------
            # per (kb, pair, half): [64, 512] psum, partition 32j+p =
            # head-in-pair j out-channel p of that half; evac copies split
            # across ScalarE and VectorE
            for kb in range(4):
                sl = slice(512 * kb, 512 * kb + 512)
                for pr, ktr in ((0, kt0_s), (1, kt1_s)):
                    for i, mq in ((0, mqlo_s), (1, mqhi_s)):
                        pj = ps_misc.tile([64, 512], f32, tag="mb",
                                          name=f"pj{kb}_{pr}_{i}")
                        nc.tensor.matmul(pj, mq, ktr[:, sl],
                                         start=True, stop=True)
                        if (kb + pr) % 2 == 0:
                            nc.vector.tensor_copy(ktd_s[pr][:, i, sl], pj)
                        else:
                            nc.scalar.mul(ktd_s[pr][:, i, sl], pj, 1.0)

            # ---------------- attention ----------------
            # deferred-work queues: AV/norm of window W and fc of window W-1
            # pop inside window W+1's kt loop
            av_q = []
            fc_q = []

            def emit_av_group(w, qb, pr, t, j, ex_tiles, state):
                # 16 accumulating AV matmuls, emitted in 4 chunks
                h = 2 * pr + j
                def chunk(c):
                    if c == 0:
                        # bank-aligned allocation; only cols 0:66 are used
                        state[(t, j)] = ps_av.tile(
                            [128, 512], f32, tag="av", name=f"o2_{w}_{t}_{j}")
                    o2 = state[(t, j)]
                    for kt in range(4 * c, 4 * c + 4):
                        nc.tensor.matmul(
                            o2[:, 0:66],
                            ex_tiles[kt][:, 512 * j + 128 * t:
                                         512 * j + 128 * t + 128],
                            v1_s[:, kt, h, 0:66],
                            start=(kt == 0), stop=(kt == NKT - 1))
                return [lambda c=c: chunk(c) for c in range(4)]

            def emit_norm(w, qb, pr, t, j, state):
                # reciprocal of the denominator column + normalize into the
                # staging tile; after head B, DMA-transpose to oT
                def norm():
                    o2 = state[(t, j)]
                    if j == 0:
                        state[("stag", t)] = stagp.tile(
                            [128, 128], bf16, tag="stag", name=f"st{w}_{t}")
                    stg = state[("stag", t)]
                    rd = rdp.tile([128, 1], f32, tag="rd",
                                  name=f"rd{w}_{t}_{j}")
                    nc.vector.reciprocal(out=rd, in_=o2[:, 64:65])
                    nc.vector.tensor_scalar(
                        out=stg[:, 64 * j:64 * j + 64], in0=o2[:, 0:64],
                        scalar1=rd, scalar2=None, op0=MULT)
                    if j == 1:
                        q0 = 512 * qb + 128 * t
                        nc.sync.dma_start_transpose(
                            out=oT_s[pr][:, q0:q0 + 128], in_=stg)
                return [norm]

            def emit_fc(st, cb):
                def fc():
                    yp = ps_misc.tile([128, 512], f32, tag="mb",
                                      name=f"yp{st}_{cb}")
                    nc.tensor.matmul(yp, oT_s[0][:, 128 * st:128 * st + 128],
                                     wf_s[:, 0, 512 * cb:512 * cb + 512],
                                     start=True, stop=False)
                    nc.tensor.matmul(yp, oT_s[1][:, 128 * st:128 * st + 128],
                                     wf_s[:, 1, 512 * cb:512 * cb + 512],
                                     start=False, stop=True)
                    if cb == 0:
                        _CACHE_ysb[st] = ysbp.tile([128, D], bf16, tag="y",
                                                   name=f"y{st}")
                    y_sb = _CACHE_ysb[st]
                    nc.vector.tensor_copy(y_sb[:, 512 * cb:512 * cb + 512],
                                          yp)
                    if cb == D // 512 - 1:
                        nc.sync.dma_start(
                            out=y_d[128 * st:128 * st + 128, :], in_=y_sb)
                return fc

            _CACHE_ysb = {}
            DVE_KTS = (2, 5, 8, 11, 14)   # fast-exp slots per window

            for w in range(2 * NQB):
                qb, pr = w // 2, w % 2
                ex_tiles = []
                for kt in range(NKT):
                    ks = slice(128 * kt, 128 * kt + 128)
                    sc = ps_sc.tile([128, 1024], f32, tag="sc",
                                    name=f"sc{w}_{kt}")
                    for j in range(2):
                        hb = 32 * j
                        nc.tensor.matmul(
                            sc[:, 512 * j:512 * j + 512],
                            ktd_s[pr][hb:hb + 32, :, ks],
                            qd_s[pr][hb:hb + 32, :, 512 * qb:512 * qb + 512],
                            start=True, stop=True, perf_mode=DR)
                    ex = expp.tile([128, 1024], bf16, tag="ex",
                                   name=f"ex{w}_{kt}")
                    ex_tiles.append(ex)
                    if kt in DVE_KTS:
                        nc.vector.tensor_scalar(
                            out=ex.bitcast(i16), in0=sc,
                            scalar1=FEXP_MUL, scalar2=FEXP_ADD,
                            op0=MULT, op1=mybir.AluOpType.add)
                    else:
                        nc.scalar.activation(out=ex, in_=sc, func=EXP,
                                             scale=ACT_SCALE)
                    # drain deferred work: ~3 AV items per slot; fc pops
                    # late in the window so the pr1 transposes land first
                    for _ in range(3):
                        if av_q:
                            av_q.pop(0)()
                    if kt >= 9 and kt % 2 == 1:
                        for _ in range(2):
                            if fc_q:
                                fc_q.pop(0)()

                # queue this window's AV + norm; order (t, head) so the
                # staging tile fills A then B, then transposes
                state = {}
                for t in range(4):
                    for j in range(2):
                        av_q += emit_av_group(w, qb, pr, t, j, ex_tiles,
                                              state)
                        av_q += emit_norm(w, qb, pr, t, j, state)
                if pr == 1:
                    for t in range(4):
                        for cb in range(D // 512):
                            fc_q.append(emit_fc(4 * qb + t, cb))

            while av_q:
                av_q.pop(0)()
            while fc_q:
                fc_q.pop(0)()

    nc.compile()
    return nc


def _prep(query, key, value, Wq, bq, Wk, bk, Wv, bv, Wfc, bfc):
    """Host-side sharding / layout prep. Returns (in_maps, bfc_eff)."""
    import ml_dtypes
    F8 = ml_dtypes.float8_e4m3
    BF = ml_dtypes.bfloat16

    query = np.asarray(query, dtype=np.float32)
    key = np.asarray(key, dtype=np.float32)
    value = np.asarray(value, dtype=np.float32)
    Wq = np.asarray(Wq, np.float32); bq = np.asarray(bq, np.float32)
    Wk = np.asarray(Wk, np.float32); bk = np.asarray(bk, np.float32)
    Wv = np.asarray(Wv, np.float32); bv = np.asarray(bv, np.float32)
    Wfc = np.asarray(Wfc, np.float32); bfc = np.asarray(bfc, np.float32)

    # q-projection folded into k: score = q . (M k), M = Wq^T Wk
    mqt = np.ascontiguousarray(Wk.T @ Wq)          # [d, c] = M^T
    z = np.zeros((HD, 32), np.float32)
    mqlo = np.block([[mqt[:, 0:32], z], [z, mqt[:, 0:32]]]).astype(F8)
    mqhi = np.block([[mqt[:, 32:64], z], [z, mqt[:, 32:64]]]).astype(F8)

    # fold Wv / bv into fc
    A = np.empty((D, D), np.float32)
    bfc_eff = bfc.copy()
    for h in range(HEAD):
        Wfc_h = Wfc[:, HD * h:HD * h + HD]
        A[:, HD * h:HD * h + HD] = Wfc_h @ Wv
        bfc_eff += Wfc_h @ bv
    At = np.ascontiguousarray(A.T)                 # [ch, c]

    # bias cross-term per-k factor: c(k) = (Wk^T bq).k + bq.bk
    u = Wk.T @ bq
    cconst = float(bq @ bk)

    qT = query.transpose(0, 2, 1)                  # [B, D, S]
    kT = key.transpose(0, 2, 1)

    in_maps = []
    for core in range(N_CORES):
        b, hg = core // 4, core % 4
        ch0 = CH * hg
        # qd_pr[32j+p, i, s] = q[b, s, ch0+128pr+64j+32i+p]
        qcore = qT[b][ch0:ch0 + CH]                # [256, S]
        qd = np.empty((2, 64, 2, S), np.float32)
        for pr in range(2):
            for j in range(2):
                for i in range(2):
                    c0 = 128 * pr + 64 * j + 32 * i
                    qd[pr, 32 * j:32 * j + 32, i] = qcore[c0:c0 + 32]
        kcore = kT[b][ch0:ch0 + CH]
        # per-k, per-head bias factor E = exp(c_h(k)/8),
        # c_h(k) = (Wk^T bq) . k[head h channels] + bq.bk
        v1 = np.zeros((128, NKT, HPC, 66), np.float32)
        for h in range(HPC):
            chh = slice(ch0 + HD * h, ch0 + HD * h + HD)
            c_h = key[b][:, chh] @ u + cconst      # [S]
            E = np.exp(c_h / 8.0).astype(np.float32)
            vh = value[b][:, chh] * E[:, None]     # [S, 64]
            for kt in range(NKT):
                rows = slice(128 * kt, 128 * kt + 128)
                v1[:, kt, h, 0:64] = vh[rows]
                v1[:, kt, h, 64] = E[rows] / S_O
        wf = np.empty((128, 2, D), np.float32)
        for pr in range(2):
            wf[:, pr, :] = At[ch0 + 128 * pr:ch0 + 128 * pr + 128] * WF_S
        in_maps.append({
            "qd0": np.ascontiguousarray(qd[0].reshape(64, 2 * S)).astype(F8),
            "qd1": np.ascontiguousarray(qd[1].reshape(64, 2 * S)).astype(F8),
            "kt0": np.ascontiguousarray(kcore[0:128]).astype(F8),
            "kt1": np.ascontiguousarray(kcore[128:256]).astype(F8),
            "mqlo": mqlo,
            "mqhi": mqhi,
            "v1": np.ascontiguousarray(
                v1.reshape(128, NKT * HPC * 66)).astype(BF),
            "wf": np.ascontiguousarray(wf.reshape(128, 2 * D)).astype(BF),
        })
    return in_maps, bfc_eff


def _run_once(inputs):
    global LAST_RESULTS
    from concourse.bass_utils import run_bass_kernel_spmd

    if "nc" not in _CACHE:
        _CACHE["nc"] = _build()
    nc = _CACHE["nc"]

    in_maps, bfc_eff = _prep(**inputs)
    res = run_bass_kernel_spmd(nc, in_maps, core_ids=list(range(N_CORES)))
    LAST_RESULTS = res

    out = np.empty((B, S, D), np.float32)
    for b in range(B):
        acc = res.results[4 * b]["y"].astype(np.float32).copy()
        for hg in range(1, 4):
            acc += res.results[4 * b + hg]["y"].astype(np.float32)
        out[b] = acc * OUT_SCALE + bfc_eff
    return out


def kernel(**inputs) -> np.ndarray:
    last_exc = None
    for attempt in range(3):
        try:
            out = _run_once(inputs)
            amax = float(np.abs(out).max())
            if np.isfinite(out).all() and 1e-6 < amax < 1e3:
                return out
            raise RuntimeError(f"implausible kernel output (absmax={amax})")
        except Exception as e:  # noqa: BLE001 - retry transient failures
            last_exc = e
            _CACHE.pop("nc", None)
            _CACHE["nonce"] = attempt + 1
    raise last_exc


# revision 19
# speedup vs baseline: 1.4517x; 1.3432x over previous
"""Multi-head attention (B=2, S=2048, D=1024, H=16) on 8 Trainium2 NeuronCores.

Sharding: core i handles batch b = i//4 and head-group hg = i%4 (4 heads,
256 channels).  The fc layer is sharded over its contraction dim; each core
emits a bf16 partial y that the host sums per batch.  Wv/bv are folded into
Wfc/bfc on the host (exact: softmax rows sum to 1).

Algebraic folds (host, weights-only):
  - q-projection is folded into the k side: score = q_raw . (M k_raw) with
    M = Wq^T Wk, so only k needs an on-device projection (k~ = M k).
  - bias cross-terms: score = (Wq q).(Wk k) + c(k) + d(q) + const; the d(q)
    and const terms cancel in softmax; exp(c(k)/8) is folded into the v/ones
    columns on the host (c(k) = (Wk^T bq).k + bq.bk; exactly 1 when bq=0).

Device pipeline per core (per-head attention, 4 heads):
  - k~ projection: fp8 raw k x fp8 block-diag M columns -> psum -> fp8 k~D
    in DoubleRow layout [32, 2, S] per head (contraction split 64 = 32x2)
  - scores: fp8 DoubleRow matmuls (2 heads per 1024-wide psum tile),
    0.5 cycles/row on the PE
  - exp: split across ScalarE (native Exp -> bf16) and VectorE (Schraudolph
    fast-exp: score*16*log2e + (16256-5.5) converted to int16, bit-cast to
    bf16); both write per-kt ex tiles [128k, 1024] bf16
  - AV transposed: o2[q,66] = sum_kt exT[k,q-tile].T @ v1[k,66] (bf16, ones
    column 64 = E/32 gives the denominator; col 65 pads to an even width)
  - normalize: per-partition reciprocal of col 64 + tensor_scalar multiply
    into a [128,128] staging tile (two heads side by side), then a DMA-XBAR
    transpose to oT[pair][ch,128q] -- which also lands head B on partitions
    64..127, making fc a plain K=128 matmul per pair
  - fc: per (st, cb): two bf16 matmuls (pair 0 start / pair 1 stop) into a
    [128,512] psum, DVE-evacuated to bf16 and DMA'd out
  - AV/norm/fc work of window W is interleaved into window W+1's kt slots so
    the PE, ScalarE and VectorE streams all stay busy
"""

import sys

import numpy as np

if "/opt/trn_rl_repo" not in sys.path:
    sys.path.insert(0, "/opt/trn_rl_repo")

HEAD = 16
B, S, D = 2, 2048, 1024
HD = 64
HPC = 4          # heads per core
CH = HPC * HD    # channels per core
N_CORES = 8
NQB = S // 512   # q windows
NKT = S // 128   # k tiles
NST = S // 128   # output row tiles

LOG2E = float(np.log2(np.e))
ACT_SCALE = 0.125           # 1/sqrt(hd) applied inside exp
FEXP_MUL = 16.0 * LOG2E     # bf16 fast-exp: bits = X*16*log2e + 16256 - sigma
FEXP_ADD = 16256.0 - 5.5
S_O = 32.0                  # oT scale (ones column = E/32)
WF_S = 16.0                 # wf scale
OUT_SCALE = 1.0 / (S_O * WF_S)

_CACHE = {}
LAST_RESULTS = None


def _build():
    import concourse.tile as tile
    from concourse import bacc, mybir

    f32 = mybir.dt.float32
    f8 = mybir.dt.float8e4
    bf16 = mybir.dt.bfloat16
    i16 = mybir.dt.int16
    EXP = mybir.ActivationFunctionType.Exp
    DR = mybir.MatmulPerfMode.DoubleRow
    MULT = mybir.AluOpType.mult

    nc = bacc.Bacc("TRN2", target_bir_lowering=False, debug=False,
                   num_devices=N_CORES)

    nonce = _CACHE.get("nonce", 0)
    if nonce:
        nc.dram_tensor(f"retry_nonce_{nonce}", [1, 1], mybir.dt.float32)

    qd0_d = nc.dram_tensor("qd0", [64, 2 * S], f8, kind="ExternalInput")
    qd1_d = nc.dram_tensor("qd1", [64, 2 * S], f8, kind="ExternalInput")
    kt0_d = nc.dram_tensor("kt0", [128, S], f8, kind="ExternalInput")
    kt1_d = nc.dram_tensor("kt1", [128, S], f8, kind="ExternalInput")
    mqlo_d = nc.dram_tensor("mqlo", [128, 64], f8, kind="ExternalInput")
    mqhi_d = nc.dram_tensor("mqhi", [128, 64], f8, kind="ExternalInput")
    v1_d = nc.dram_tensor("v1", [128, NKT * HPC * 66], bf16,
                          kind="ExternalInput")
    wf_d = nc.dram_tensor("wf", [128, 2 * D], bf16, kind="ExternalInput")
    y_d = nc.dram_tensor("y", [S, D], bf16, kind="ExternalOutput")

    with tile.TileContext(nc) as tc, nc.allow_low_precision(
            reason="fp8/bf16 matmul inputs; psum accumulation stays f32"):
        with (
            tc.tile_pool(name="consts", bufs=1) as consts,
            tc.tile_pool(name="exp", bufs=34) as expp,
            tc.tile_pool(name="stag", bufs=4) as stagp,
            tc.tile_pool(name="rd", bufs=4) as rdp,
            tc.tile_pool(name="ysb", bufs=3) as ysbp,
            tc.tile_pool(name="ps_sc", bufs=3, space="PSUM") as ps_sc,
            tc.tile_pool(name="ps_misc", bufs=2, space="PSUM") as ps_misc,
        ):
            # ---------------- input tiles ----------------
            mqlo_s = consts.tile([128, 64], f8, tag="mqlo")
            mqhi_s = consts.tile([128, 64], f8, tag="mqhi")
            kt0_s = consts.tile([128, S], f8, tag="kt0")
            kt1_s = consts.tile([128, S], f8, tag="kt1")
            qd_s = [consts.tile([64, 2, S], f8, tag=f"qd{pr}",
                                name=f"qd{pr}") for pr in range(2)]
            v1_s = consts.tile([128, NKT, HPC, 66], bf16, tag="v1")
            wf_s = consts.tile([128, 2, D], bf16, tag="wf")
            ktd_s = [consts.tile([64, 2, S], f8, tag=f"ktd{pr}",
                                 name=f"ktd{pr}") for pr in range(2)]
            oT_s = [consts.tile([128, S], bf16, tag=f"oT{pr}",
                                name=f"oT{pr}") for pr in range(2)]

            # critical-path first: proj weights + k chunks, then q, v, wf
            nc.sync.dma_start(out=mqlo_s, in_=mqlo_d[:, :])
            nc.sync.dma_start(out=mqhi_s, in_=mqhi_d[:, :])
            for c in range(4):
                sl = slice(512 * c, 512 * c + 512)
                nc.sync.dma_start(out=kt0_s[:, sl], in_=kt0_d[:, sl])
                nc.sync.dma_start(out=kt1_s[:, sl], in_=kt1_d[:, sl])
            nc.sync.dma_start(out=qd_s[0], in_=qd0_d[:, :])
            nc.sync.dma_start(out=qd_s[1], in_=qd1_d[:, :])
            nc.sync.dma_start(out=v1_s, in_=v1_d[:, :])
            nc.sync.dma_start(out=wf_s, in_=wf_d[:, :])

            # ---------------- k~ projection ----------------
            # per (kb, pair, half): [64, 512] psum, partition 32j+p =
            # head-in-pair j out-channel p of that half; evac copies split
            # across ScalarE and VectorE
            for kb in range(4):
                sl = slice(512 * kb, 512 * kb + 512)
                for pr, ktr in ((0, kt0_s), (1, kt1_s)):
                    for i, mq in ((0, mqlo_s), (1, mqhi_s)):
                        pj = ps_misc.tile([64, 512], f32, tag="mb",
                                          name=f"pj{kb}_{pr}_{i}")
                        nc.tensor.matmul(pj, mq, ktr[:, sl],
                                         start=True, stop=True)
                        if (kb + pr) % 2 == 0:
                            nc.vector.tensor_copy(ktd_s[pr][:, i, sl], pj)
                        else:
                            nc.scalar.mul(ktd_s[pr][:, i, sl], pj, 1.0)

            # ---------------- attention ----------------
            # deferred-work queues: AV/norm of window W and fc of window W-1
            # pop inside window W+1's kt loop
            av_q = []
            fc_q = []

            def emit_av_group(w, qb, pr, t, j, ex_tiles, state):
                # 16 accumulating AV matmuls, emitted in 4 chunks
                h = 2 * pr + j
                def chunk(c):
                    if c == 0:
                        # bank-aligned allocation; only cols 0:66 are used
                        state[(t, j)] = ps_misc.tile(
                            [128, 512], f32, tag="mb", name=f"o2_{w}_{t}_{j}")
                    o2 = state[(t, j)]
                    for kt in range(4 * c, 4 * c + 4):
                        nc.tensor.matmul(
                            o2[:, 0:66],
                            ex_tiles[kt][:, 512 * j + 128 * t:
                                         512 * j + 128 * t + 128],
                            v1_s[:, kt, h, 0:66],
                            start=(kt == 0), stop=(kt == NKT - 1))
                return [lambda c=c: chunk(c) for c in range(4)]

            def emit_norm(w, qb, pr, t, j, state):
                # reciprocal of the denominator column + normalize into the
                # staging tile; after head B, DMA-transpose to oT
                def norm():
                    o2 = state[(t, j)]
                    if j == 0:
                        state[("stag", t)] = stagp.tile(
                            [128, 128], bf16, tag="stag", name=f"st{w}_{t}")
                    stg = state[("stag", t)]
                    rd = rdp.tile([128, 1], f32, tag="rd",
                                  name=f"rd{w}_{t}_{j}")
                    nc.vector.reciprocal(out=rd, in_=o2[:, 64:65])
                    nc.vector.tensor_scalar(
                        out=stg[:, 64 * j:64 * j + 64], in0=o2[:, 0:64],
                        scalar1=rd, scalar2=None, op0=MULT)
                    if j == 1:
                        q0 = 512 * qb + 128 * t
                        nc.sync.dma_start_transpose(
                            out=oT_s[pr][:, q0:q0 + 128], in_=stg)
                return [norm]

            def emit_fc(st, cb):
                def fc():
                    yp = ps_misc.tile([128, 512], f32, tag="mb",
                                      name=f"yp{st}_{cb}")
                    nc.tensor.matmul(yp, oT_s[0][:, 128 * st:128 * st + 128],
                                     wf_s[:, 0, 512 * cb:512 * cb + 512],
                                     start=True, stop=False)
                    nc.tensor.matmul(yp, oT_s[1][:, 128 * st:128 * st + 128],
                                     wf_s[:, 1, 512 * cb:512 * cb + 512],
                                     start=False, stop=True)
                    if cb == 0:
                        _CACHE_ysb[st] = ysbp.tile([128, D], bf16, tag="y",
                                                   name=f"y{st}")
                    y_sb = _CACHE_ysb[st]
                    nc.vector.tensor_copy(y_sb[:, 512 * cb:512 * cb + 512],
                                          yp)
                    if cb == D // 512 - 1:
                        nc.sync.dma_start(
                            out=y_d[128 * st:128 * st + 128, :], in_=y_sb)
                return fc

            _CACHE_ysb = {}
            # 44 of 128 exps on the DVE (fast-exp), the rest on ScalarE
            DVE_KTS_EVEN = (2, 5, 8, 11, 14)
            DVE_KTS_ODD = (1, 4, 7, 9, 12, 15)

            for w in range(2 * NQB):
                qb, pr = w // 2, w % 2
                dve_kts = DVE_KTS_EVEN if w % 2 == 0 else DVE_KTS_ODD
                ex_tiles = []
                for kt in range(NKT):
                    ks = slice(128 * kt, 128 * kt + 128)
                    sc = ps_sc.tile([128, 1024], f32, tag="sc",
                                    name=f"sc{w}_{kt}")
                    for j in range(2):
                        hb = 32 * j
                        nc.tensor.matmul(
                            sc[:, 512 * j:512 * j + 512],
                            ktd_s[pr][hb:hb + 32, :, ks],
                            qd_s[pr][hb:hb + 32, :, 512 * qb:512 * qb + 512],
                            start=True, stop=True, perf_mode=DR)
                    ex = expp.tile([128, 1024], bf16, tag="ex",
                                   name=f"ex{w}_{kt}")
                    ex_tiles.append(ex)
                    if kt in dve_kts:
                        nc.vector.tensor_scalar(
                            out=ex.bitcast(i16), in0=sc,
                            scalar1=FEXP_MUL, scalar2=FEXP_ADD,
                            op0=MULT, op1=mybir.AluOpType.add)
                    else:
                        nc.scalar.activation(out=ex, in_=sc, func=EXP,
                                             scale=ACT_SCALE)
                    # drain deferred work: ~4 AV items per slot; fc pops
                    # late in the window so the pr1 transposes land first
                    for _ in range(4):
                        if av_q:
                            av_q.pop(0)()
                    if kt >= 9 and kt % 2 == 1:
                        for _ in range(2):
                            if fc_q:
                                fc_q.pop(0)()

                # queue this window's AV + norm; order (t, head) so the
                # staging tile fills A then B, then transposes
                state = {}
                for t in range(4):
                    for j in range(2):
                        av_q += emit_av_group(w, qb, pr, t, j, ex_tiles,
                                              state)
                        av_q += emit_norm(w, qb, pr, t, j, state)
                if pr == 1:
                    for t in range(4):
                        for cb in range(D // 512):
                            fc_q.append(emit_fc(4 * qb + t, cb))

            while av_q:
                av_q.pop(0)()
            while fc_q:
                fc_q.pop(0)()

    nc.compile()
    return nc


def _prep(query, key, value, Wq, bq, Wk, bk, Wv, bv, Wfc, bfc):
    """Host-side sharding / layout prep. Returns (in_maps, bfc_eff)."""
    import ml_dtypes
    F8 = ml_dtypes.float8_e4m3
    BF = ml_dtypes.bfloat16

    query = np.asarray(query, dtype=np.float32)
    key = np.asarray(key, dtype=np.float32)
    value = np.asarray(value, dtype=np.float32)
    Wq = np.asarray(Wq, np.float32); bq = np.asarray(bq, np.float32)
    Wk = np.asarray(Wk, np.float32); bk = np.asarray(bk, np.float32)
    Wv = np.asarray(Wv, np.float32); bv = np.asarray(bv, np.float32)
    Wfc = np.asarray(Wfc, np.float32); bfc = np.asarray(bfc, np.float32)

    # q-projection folded into k: score = q . (M k), M = Wq^T Wk
    mqt = np.ascontiguousarray(Wk.T @ Wq)          # [d, c] = M^T
    z = np.zeros((HD, 32), np.float32)
    mqlo = np.block([[mqt[:, 0:32], z], [z, mqt[:, 0:32]]]).astype(F8)
    mqhi = np.block([[mqt[:, 32:64], z], [z, mqt[:, 32:64]]]).astype(F8)

    # fold Wv / bv into fc
    A = np.empty((D, D), np.float32)
    bfc_eff = bfc.copy()
    for h in range(HEAD):
        Wfc_h = Wfc[:, HD * h:HD * h + HD]
        A[:, HD * h:HD * h + HD] = Wfc_h @ Wv
        bfc_eff += Wfc_h @ bv
    At = np.ascontiguousarray(A.T)                 # [ch, c]

    # bias cross-term per-k factor: c(k) = (Wk^T bq).k + bq.bk
    u = Wk.T @ bq
    cconst = float(bq @ bk)

    qT = query.transpose(0, 2, 1)                  # [B, D, S]
    kT = key.transpose(0, 2, 1)

    in_maps = []
    for core in range(N_CORES):
        b, hg = core // 4, core % 4
        ch0 = CH * hg
        # qd_pr[32j+p, i, s] = q[b, s, ch0+128pr+64j+32i+p]
        qcore = qT[b][ch0:ch0 + CH]                # [256, S]
        qd = np.empty((2, 64, 2, S), np.float32)
        for pr in range(2):
            for j in range(2):
                for i in range(2):
                    c0 = 128 * pr + 64 * j + 32 * i
                    qd[pr, 32 * j:32 * j + 32, i] = qcore[c0:c0 + 32]
        kcore = kT[b][ch0:ch0 + CH]
        # per-k, per-head bias factor E = exp(c_h(k)/8),
        # c_h(k) = (Wk^T bq) . k[head h channels] + bq.bk
        v1 = np.zeros((128, NKT, HPC, 66), np.float32)
        for h in range(HPC):
            chh = slice(ch0 + HD * h, ch0 + HD * h + HD)
            c_h = key[b][:, chh] @ u + cconst      # [S]
            E = np.exp(c_h / 8.0).astype(np.float32)
            vh = value[b][:, chh] * E[:, None]     # [S, 64]
            for kt in range(NKT):
                rows = slice(128 * kt, 128 * kt + 128)
                v1[:, kt, h, 0:64] = vh[rows]
                v1[:, kt, h, 64] = E[rows] / S_O
        wf = np.empty((128, 2, D), np.float32)
        for pr in range(2):
            wf[:, pr, :] = At[ch0 + 128 * pr:ch0 + 128 * pr + 128] * WF_S
        in_maps.append({
            "qd0": np.ascontiguousarray(qd[0].reshape(64, 2 * S)).astype(F8),
            "qd1": np.ascontiguousarray(qd[1].reshape(64, 2 * S)).astype(F8),
            "kt0": np.ascontiguousarray(kcore[0:128]).astype(F8),
            "kt1": np.ascontiguousarray(kcore[128:256]).astype(F8),
            "mqlo": mqlo,
            "mqhi": mqhi,
            "v1": np.ascontiguousarray(
                v1.reshape(128, NKT * HPC * 66)).astype(BF),
            "wf": np.ascontiguousarray(wf.reshape(128, 2 * D)).astype(BF),
        })
    return in_maps, bfc_eff


def _run_once(inputs):
    global LAST_RESULTS
    from concourse.bass_utils import run_bass_kernel_spmd

    if "nc" not in _CACHE:
        _CACHE["nc"] = _build()
    nc = _CACHE["nc"]

    in_maps, bfc_eff = _prep(**inputs)
    res = run_bass_kernel_spmd(nc, in_maps, core_ids=list(range(N_CORES)))
    LAST_RESULTS = res

    out = np.empty((B, S, D), np.float32)
    for b in range(B):
        acc = res.results[4 * b]["y"].astype(np.float32).copy()
        for hg in range(1, 4):
            acc += res.results[4 * b + hg]["y"].astype(np.float32)
        out[b] = acc * OUT_SCALE + bfc_eff
    return out


def kernel(**inputs) -> np.ndarray:
    last_exc = None
    for attempt in range(3):
        try:
            out = _run_once(inputs)
            amax = float(np.abs(out).max())
            if np.isfinite(out).all() and 1e-6 < amax < 1e3:
                return out
            raise RuntimeError(f"implausible kernel output (absmax={amax})")
        except Exception as e:  # noqa: BLE001 - retry transient failures
            last_exc = e
            _CACHE.pop("nc", None)
            _CACHE["nonce"] = attempt + 1
    raise last_exc
